# revision 1
# baseline (speedup 1.0000x reference)
"""Single-head attention kernel for Trainium2 (8 NeuronCores, SPMD).

Problem: x[4,4096,1024] f32, padding_mask[4,1,4096] i32, Wk/Wq/Wv[64,1024] f32.
  k/q/v = x @ W.T ; wei = softmax(mask(q k^T / 8)) ; out = wei @ v  -> [4,4096,64]

Sharding: core c handles (batch b = c//2, query half = c%2).  Each core gets the
full x[b] (rotated so its 2048 local queries are always rows 0:2048 -- attention
is permutation-invariant over keys, so rotating keys + key-mask together is
exact), computes k/v for all 4096 keys and q for its half, and returns
out[2048, 64].

Device algorithm (per core):
  Phase 1: transpose x via PE matmul-with-identity (contraction needs C on
    partitions), then project kT/qT/vT = W.T^T @ xT in [H, T] layout; vT is
    re-transposed into v[keys, 64] and extended with a ones column (col 64)
    used to compute softmax denominators via the second matmul.
  Phase 2: per 512-query block: sT[keys,queries] = kT_chunk.T @ qT (PSUM),
    exp via ScalarE with per-partition bias = -1e5*(1-key_mask) (masked keys
    underflow to exactly 0; no row-max subtraction needed since scores are
    O(5)), then oT[65, 512] += v_ext.T @ exp accumulated over key chunks.
    Epilogue: transpose oT back, scale rows by query_mask/denominator.
"""

import sys

if "/opt/trn_rl_repo" not in sys.path:
    sys.path.insert(0, "/opt/trn_rl_repo")

import numpy as np

import concourse.bass as bass
import concourse.mybir as mybir
import concourse.tile as tile
from concourse import bacc
from concourse.bass_utils import run_bass_kernel_spmd

F32 = mybir.dt.float32
T = 4096          # sequence length (keys)
C = 1024          # embedding dim
H = 64            # head size
TBS = 512         # t-block size for phase 1
NTB = T // TBS    # 8 t-blocks
NCC = C // 128    # 8 c-chunks
QL = 2048         # local queries per core
NQB = QL // 512   # 4 query blocks
NKC = T // 128    # 32 key chunks
NEG = -1.0e5      # masked-key bias: exp(s/8 + NEG) underflows to 0.0


def build_nc():
    nc = bacc.Bacc("TRN2", target_bir_lowering=False, debug=False, num_devices=8)

    x_d = nc.dram_tensor("x", [T, C], F32, kind="ExternalInput")
    wkt_d = nc.dram_tensor("wkt", [128, NCC, H], F32, kind="ExternalInput")
    wqt_d = nc.dram_tensor("wqt", [128, NCC, H], F32, kind="ExternalInput")
    wvt_d = nc.dram_tensor("wvt", [128, NCC, H], F32, kind="ExternalInput")
    ident_d = nc.dram_tensor("ident", [128, 128], F32, kind="ExternalInput")
    nbias_d = nc.dram_tensor("nbias", [128, NKC], F32, kind="ExternalInput")
    maskq_d = nc.dram_tensor("maskq", [128, QL // 128], F32, kind="ExternalInput")
    out_d = nc.dram_tensor("out", [QL, H], F32, kind="ExternalOutput")

    with tile.TileContext(nc) as tc:
        with (
            tc.tile_pool(name="const", bufs=1) as const,
            tc.tile_pool(name="persist", bufs=1) as persist,
            tc.tile_pool(name="xin", bufs=2) as xin,
            tc.tile_pool(name="xt", bufs=3) as xtp,
            tc.tile_pool(name="vt", bufs=2) as vtp,
            tc.tile_pool(name="expp", bufs=4) as expp,
            tc.tile_pool(name="osb", bufs=2) as osb,
            tc.tile_pool(name="small", bufs=4) as small,
            tc.tile_pool(name="psA", bufs=2, space=bass.MemorySpace.PSUM) as psA,
            tc.tile_pool(name="psB", bufs=1, space=bass.MemorySpace.PSUM) as psB,
            tc.tile_pool(name="psC", bufs=2, space=bass.MemorySpace.PSUM) as psC,
            tc.tile_pool(name="psD", bufs=1, space=bass.MemorySpace.PSUM) as psD,
        ):
            # ---- constants ----
            wkt_sb = const.tile([128, NCC, H], F32)
            wqt_sb = const.tile([128, NCC, H], F32)
            wvt_sb = const.tile([128, NCC, H], F32)
            ident_sb = const.tile([128, 128], F32)
            nbias_sb = const.tile([128, NKC], F32)
            maskq_sb = const.tile([128, QL // 128], F32)
            nc.sync.dma_start(out=wkt_sb, in_=wkt_d.ap())
            nc.sync.dma_start(out=wqt_sb, in_=wqt_d.ap())
            nc.sync.dma_start(out=wvt_sb, in_=wvt_d.ap())
            nc.sync.dma_start(out=ident_sb, in_=ident_d.ap())
            nc.sync.dma_start(out=nbias_sb, in_=nbias_d.ap())
            nc.sync.dma_start(out=maskq_sb, in_=maskq_d.ap())

            # ---- persistent intermediates ----
            kT_sb = persist.tile([H, T], F32)          # k^T  [64, 4096]
            qT_sb = persist.tile([H, QL], F32)         # q^T  [64, 2048]
            v_sb = persist.tile([128, NKC, H + 1], F32)  # v_ext [keys, 65]
            out_acc = persist.tile([128, QL // 128, H], F32)
            nc.vector.memset(v_sb[:, :, H : H + 1], 1.0)  # ones column

            # ================= Phase 1: transpose + projections ============
            for tb in range(NTB):
                x_tile = xin.tile([128, TBS // 128, C], F32)
                nc.sync.dma_start(
                    out=x_tile,
                    in_=x_d.ap()[tb * TBS : (tb + 1) * TBS, :].rearrange(
                        "(s p) c -> p s c", p=128
                    ),
                )
                kqv_ps = psB.tile([H, 3, TBS], F32)
                for cc in range(NCC):
                    # transpose x[tb, cc]: 4x [128t,128c] -> [128c, 512t]
                    tp_ps = psA.tile([128, TBS], F32, tag="pa")
                    for s in range(TBS // 128):
                        nc.tensor.matmul(
                            tp_ps[:, s * 128 : (s + 1) * 128],
                            x_tile[:, s, cc * 128 : (cc + 1) * 128],
                            ident_sb,
                            start=True,
                            stop=True,
                        )
                    xT_sb = xtp.tile([128, TBS], F32)
                    # alternate ACT/DVE for PSUM->SBUF copies to split the load
                    if cc % 2 == 0:
                        nc.scalar.copy(xT_sb, tp_ps)
                    else:
                        nc.vector.tensor_copy(xT_sb, tp_ps)
                    first, last = cc == 0, cc == NCC - 1
                    nc.tensor.matmul(
                        kqv_ps[:, 0, :],
                        wkt_sb[:, cc, :],
                        xT_sb,
                        start=first,
                        stop=last,
                    )
                    nc.tensor.matmul(
                        kqv_ps[:, 1, :],
                        wvt_sb[:, cc, :],
                        xT_sb,
                        start=first,
                        stop=last,
                    )
                    if tb < NQB:  # local queries are always rows 0:2048
                        nc.tensor.matmul(
                            kqv_ps[:, 2, :],
                            wqt_sb[:, cc, :],
                            xT_sb,
                            start=first,
                            stop=last,
                        )
                nc.vector.tensor_copy(kT_sb[:, tb * TBS : (tb + 1) * TBS], kqv_ps[:, 0, :])
                if tb < NQB:
                    nc.vector.tensor_copy(
                        qT_sb[:, tb * TBS : (tb + 1) * TBS], kqv_ps[:, 2, :]
                    )
                # vT -> v (re-transpose to [keys, 64] layout)
                vT_sb = vtp.tile([H, TBS], F32)
                nc.scalar.copy(vT_sb, kqv_ps[:, 1, :])
                vtp_ps = psC.tile([128, TBS // 128, H], F32, tag="small")
                for s in range(TBS // 128):
                    nc.tensor.matmul(
                        vtp_ps[:, s, :],
                        vT_sb[:, s * 128 : (s + 1) * 128],
                        ident_sb[:H, :H],
                        start=True,
                        stop=True,
                    )
                nc.vector.tensor_copy(
                    v_sb[:, tb * (TBS // 128) : (tb + 1) * (TBS // 128), 0:H], vtp_ps
                )

            # ================= Phase 2: attention =========================
            for qb in range(NQB):
                oT_ps = psD.tile([H + 1, 512], F32)
                for kc in range(NKC):
                    sT_ps = psA.tile([128, 512], F32, tag="pa")
                    nc.tensor.matmul(
                        sT_ps,
                        kT_sb[:, kc * 128 : (kc + 1) * 128],
                        qT_sb[:, qb * 512 : (qb + 1) * 512],
                        start=True,
                        stop=True,
                    )
                    exp_sb = expp.tile([128, 512], F32)
                    nc.scalar.activation(
                        exp_sb,
                        sT_ps,
                        mybir.ActivationFunctionType.Exp,
                        bias=nbias_sb[:, kc : kc + 1],
                        scale=0.125,
                    )
                    nc.tensor.matmul(
                        oT_ps,
                        v_sb[:, kc, :],
                        exp_sb,
                        start=(kc == 0),
                        stop=(kc == NKC - 1),
                    )
                oT_sb = osb.tile([H + 1, 512], F32)
                nc.vector.tensor_copy(oT_sb, oT_ps)
                for qs in range(4):
                    qt = qb * 4 + qs
                    ot_ps = psC.tile([128, H + 1], F32, tag="small")
                    nc.tensor.matmul(
                        ot_ps,
                        oT_sb[:, qs * 128 : (qs + 1) * 128],
                        ident_sb[: H + 1, : H + 1],
                        start=True,
                        stop=True,
                    )
                    recip_sb = small.tile([128, 1], F32)
                    nc.vector.reciprocal(recip_sb, ot_ps[:, H : H + 1])
                    nc.vector.tensor_scalar(
                        out=out_acc[:, qt, :],
                        in0=ot_ps[:, 0:H],
                        scalar1=recip_sb,
                        scalar2=maskq_sb[:, qt : qt + 1],
                        op0=mybir.AluOpType.mult,
                        op1=mybir.AluOpType.mult,
                    )
            nc.sync.dma_start(
                out=out_d.ap().rearrange("(n p) h -> p n h", p=128), in_=out_acc
            )

    nc.compile()
    return nc


_NC_CACHE = None


def _get_nc():
    global _NC_CACHE
    if _NC_CACHE is None:
        _NC_CACHE = build_nc()
    return _NC_CACHE


def _prep_core_inputs(x, padding_mask, wkt, wqt, wvt, ident, core):
    b, half = core // 2, core % 2
    q0 = half * QL
    xb = x[b]
    m = padding_mask[b, 0].astype(np.float32)
    if half:  # rotate keys so local queries are rows 0:2048 (exact: permutation
        # of keys with identically-permuted key mask leaves attention unchanged)
        xb = np.concatenate([xb[q0:], xb[:q0]], axis=0)
        m = np.concatenate([m[q0:], m[:q0]], axis=0)
    nbias = np.ascontiguousarray((NEG * (1.0 - m)).reshape(NKC, 128).T)
    maskq = np.ascontiguousarray(m[:QL].reshape(QL // 128, 128).T)
    return {
        "x": np.ascontiguousarray(xb),
        "wkt": wkt,
        "wqt": wqt,
        "wvt": wvt,
        "ident": ident,
        "nbias": nbias,
        "maskq": maskq,
    }


def make_in_maps(x, padding_mask, Wk, Wq, Wv):
    def wt(w):  # [64,1024] -> [128, 8, 64]: wt[p, cc, h] = w[h, cc*128+p]
        return np.ascontiguousarray(w.T.reshape(NCC, 128, H).transpose(1, 0, 2))

    wkt, wqt, wvt = wt(np.asarray(Wk)), wt(np.asarray(Wq)), wt(np.asarray(Wv))
    ident = np.eye(128, dtype=np.float32)
    x = np.asarray(x)
    padding_mask = np.asarray(padding_mask)
    return [
        _prep_core_inputs(x, padding_mask, wkt, wqt, wvt, ident, c) for c in range(8)
    ]


def kernel(x, padding_mask, Wk, Wq, Wv):
    nc = _get_nc()
    in_maps = make_in_maps(x, padding_mask, Wk, Wq, Wv)
    res = run_bass_kernel_spmd(nc, in_maps, core_ids=list(range(8)), trace=False)
    B = x.shape[0]
    out = np.empty((B, T, H), dtype=np.float32)
    for c in range(8):
        b, half = c // 2, c % 2
        out[b, half * QL : (half + 1) * QL, :] = res.results[c]["out"]
    return out


# revision 4
# speedup vs baseline: 133.6980x; 133.6980x over previous
"""Single-head attention kernel for Trainium2 (8 NeuronCores, SPMD).

Problem: x[4,4096,1024] f32, padding_mask[4,1,4096] i32, Wk/Wq/Wv[64,1024] f32.
  k/q/v = x @ W.T ; wei = softmax(mask(q k^T / 8)) ; out = wei @ v  -> [4,4096,64]

Sharding: core c handles (batch b = c//2, query half = c%2).  Each core gets the
full x[b] (rotated so its 2048 local queries are always rows 0:2048 -- attention
is permutation-invariant over keys, so rotating keys + key-mask together is
exact), computes k/v for all 4096 keys and q for its half, and returns
out[2048, 64].

Device algorithm (per core):
  Phase 1: transpose x via PE matmul-with-identity (contraction needs C on
    partitions), then project kT/qT/vT = W.T^T @ xT in [H, T] layout; vT is
    re-transposed into v[keys, 64] and extended with a ones column (col 64)
    used to compute softmax denominators via the second matmul.
  Phase 2: per 512-query block: sT[keys,queries] = kT_chunk.T @ qT (PSUM),
    exp via ScalarE with per-partition bias = -1e5*(1-key_mask) (masked keys
    underflow to exactly 0; no row-max subtraction needed since scores are
    O(5)), then oT[65, 512] += v_ext.T @ exp accumulated over key chunks.
    Epilogue: transpose oT back, scale rows by query_mask/denominator.
"""

import sys

if "/opt/trn_rl_repo" not in sys.path:
    sys.path.insert(0, "/opt/trn_rl_repo")

import numpy as np

import concourse.bass as bass
import concourse.mybir as mybir
import concourse.tile as tile
from concourse import bacc
from concourse.bass_utils import run_bass_kernel_spmd

F32 = mybir.dt.float32
T = 4096          # sequence length (keys)
C = 1024          # embedding dim
H = 64            # head size
TBS = 512         # t-block size for phase 1
NTB = T // TBS    # 8 t-blocks
NCC = C // 128    # 8 c-chunks
QL = 2048         # local queries per core
NQB = QL // 512   # 4 query blocks
NKC = T // 128    # 32 key chunks
NEG = -1.0e5      # masked-key bias: exp(s/8 + NEG) underflows to 0.0


def build_nc(reps=1):
    nc = bacc.Bacc("TRN2", target_bir_lowering=False, debug=False, num_devices=8)

    x_d = nc.dram_tensor("x", [T, C], F32, kind="ExternalInput")
    wkt_d = nc.dram_tensor("wkt", [128, NCC, H], F32, kind="ExternalInput")
    wqt_d = nc.dram_tensor("wqt", [128, NCC, H], F32, kind="ExternalInput")
    wvt_d = nc.dram_tensor("wvt", [128, NCC, H], F32, kind="ExternalInput")
    ident_d = nc.dram_tensor("ident", [128, 128], F32, kind="ExternalInput")
    nbias_d = nc.dram_tensor("nbias", [128, NKC], F32, kind="ExternalInput")
    maskq_d = nc.dram_tensor("maskq", [128, QL // 128], F32, kind="ExternalInput")
    out_d = nc.dram_tensor("out", [QL, H], F32, kind="ExternalOutput")

    with tile.TileContext(nc) as tc:
        with (
            tc.tile_pool(name="const", bufs=1) as const,
            tc.tile_pool(name="persist", bufs=1) as persist,
            tc.tile_pool(name="xin", bufs=2) as xin,
            tc.tile_pool(name="xt", bufs=3) as xtp,
            tc.tile_pool(name="vt", bufs=2) as vtp,
            tc.tile_pool(name="expp", bufs=4) as expp,
            tc.tile_pool(name="osb", bufs=2) as osb,
            tc.tile_pool(name="small", bufs=4) as small,
            tc.tile_pool(name="psA", bufs=2, space=bass.MemorySpace.PSUM) as psA,
            tc.tile_pool(name="psB", bufs=1, space=bass.MemorySpace.PSUM) as psB,
            tc.tile_pool(name="psC", bufs=2, space=bass.MemorySpace.PSUM) as psC,
            tc.tile_pool(name="psD", bufs=1, space=bass.MemorySpace.PSUM) as psD,
        ):
            def emit_body():
                _emit(nc, tc, const, persist, xin, xtp, vtp, expp, osb, small,
                      psA, psB, psC, psD,
                      x_d, wkt_d, wqt_d, wvt_d, ident_d, nbias_d, maskq_d, out_d)

            if reps == 1:
                emit_body()
            else:
                with tc.For_i(0, reps):
                    emit_body()

    nc.compile()
    return nc


def _emit(nc, tc, const, persist, xin, xtp, vtp, expp, osb, small,
          psA, psB, psC, psD,
          x_d, wkt_d, wqt_d, wvt_d, ident_d, nbias_d, maskq_d, out_d):
            # ---- constants ----
            wkt_sb = const.tile([128, NCC, H], F32)
            wqt_sb = const.tile([128, NCC, H], F32)
            wvt_sb = const.tile([128, NCC, H], F32)
            ident_sb = const.tile([128, 128], F32)
            nbias_sb = const.tile([128, NKC], F32)
            maskq_sb = const.tile([128, QL // 128], F32)
            nc.sync.dma_start(out=wkt_sb, in_=wkt_d.ap())
            nc.sync.dma_start(out=wqt_sb, in_=wqt_d.ap())
            nc.sync.dma_start(out=wvt_sb, in_=wvt_d.ap())
            nc.sync.dma_start(out=ident_sb, in_=ident_d.ap())
            nc.sync.dma_start(out=nbias_sb, in_=nbias_d.ap())
            nc.sync.dma_start(out=maskq_sb, in_=maskq_d.ap())

            # ---- persistent intermediates ----
            kT_sb = persist.tile([H, T], F32)          # k^T  [64, 4096]
            qT_sb = persist.tile([H, QL], F32)         # q^T  [64, 2048]
            v_sb = persist.tile([128, NKC, H + 1], F32)  # v_ext [keys, 65]
            out_acc = persist.tile([128, QL // 128, H], F32)
            nc.vector.memset(v_sb[:, :, H : H + 1], 1.0)  # ones column

            # ================= Phase 1: transpose + projections ============
            for tb in range(NTB):
                x_tile = xin.tile([128, TBS // 128, C], F32)
                nc.sync.dma_start(
                    out=x_tile,
                    in_=x_d.ap()[tb * TBS : (tb + 1) * TBS, :].rearrange(
                        "(s p) c -> p s c", p=128
                    ),
                )
                kqv_ps = psB.tile([H, 3, TBS], F32)
                for cc in range(NCC):
                    # transpose x[tb, cc]: 4x [128t,128c] -> [128c, 512t]
                    tp_ps = psA.tile([128, TBS], F32, tag="pa")
                    for s in range(TBS // 128):
                        nc.tensor.matmul(
                            tp_ps[:, s * 128 : (s + 1) * 128],
                            x_tile[:, s, cc * 128 : (cc + 1) * 128],
                            ident_sb,
                            start=True,
                            stop=True,
                        )
                    xT_sb = xtp.tile([128, TBS], F32)
                    # alternate ACT/DVE for PSUM->SBUF copies to split the load
                    if cc % 2 == 0:
                        nc.scalar.copy(xT_sb, tp_ps)
                    else:
                        nc.vector.tensor_copy(xT_sb, tp_ps)
                    first, last = cc == 0, cc == NCC - 1
                    nc.tensor.matmul(
                        kqv_ps[:, 0, :],
                        wkt_sb[:, cc, :],
                        xT_sb,
                        start=first,
                        stop=last,
                    )
                    nc.tensor.matmul(
                        kqv_ps[:, 1, :],
                        wvt_sb[:, cc, :],
                        xT_sb,
                        start=first,
                        stop=last,
                    )
                    if tb < NQB:  # local queries are always rows 0:2048
                        nc.tensor.matmul(
                            kqv_ps[:, 2, :],
                            wqt_sb[:, cc, :],
                            xT_sb,
                            start=first,
                            stop=last,
                        )
                nc.vector.tensor_copy(kT_sb[:, tb * TBS : (tb + 1) * TBS], kqv_ps[:, 0, :])
                if tb < NQB:
                    nc.vector.tensor_copy(
                        qT_sb[:, tb * TBS : (tb + 1) * TBS], kqv_ps[:, 2, :]
                    )
                # vT -> v (re-transpose to [keys, 64] layout)
                vT_sb = vtp.tile([H, TBS], F32)
                nc.scalar.copy(vT_sb, kqv_ps[:, 1, :])
                vtp_ps = psC.tile([128, TBS // 128, H], F32, tag="small")
                for s in range(TBS // 128):
                    nc.tensor.matmul(
                        vtp_ps[:, s, :],
                        vT_sb[:, s * 128 : (s + 1) * 128],
                        ident_sb[:H, :H],
                        start=True,
                        stop=True,
                    )
                nc.vector.tensor_copy(
                    v_sb[:, tb * (TBS // 128) : (tb + 1) * (TBS // 128), 0:H], vtp_ps
                )

            # ================= Phase 2: attention =========================
            for qb in range(NQB):
                oT_ps = psD.tile([H + 1, 512], F32)
                for kc in range(NKC):
                    sT_ps = psA.tile([128, 512], F32, tag="pa")
                    nc.tensor.matmul(
                        sT_ps,
                        kT_sb[:, kc * 128 : (kc + 1) * 128],
                        qT_sb[:, qb * 512 : (qb + 1) * 512],
                        start=True,
                        stop=True,
                    )
                    exp_sb = expp.tile([128, 512], F32)
                    nc.scalar.activation(
                        exp_sb,
                        sT_ps,
                        mybir.ActivationFunctionType.Exp,
                        bias=nbias_sb[:, kc : kc + 1],
                        scale=0.125,
                    )
                    nc.tensor.matmul(
                        oT_ps,
                        v_sb[:, kc, :],
                        exp_sb,
                        start=(kc == 0),
                        stop=(kc == NKC - 1),
                    )
                oT_sb = osb.tile([H + 1, 512], F32)
                nc.vector.tensor_copy(oT_sb, oT_ps)
                for qs in range(4):
                    qt = qb * 4 + qs
                    ot_ps = psC.tile([128, H + 1], F32, tag="small")
                    nc.tensor.matmul(
                        ot_ps,
                        oT_sb[:, qs * 128 : (qs + 1) * 128],
                        ident_sb[: H + 1, : H + 1],
                        start=True,
                        stop=True,
                    )
                    recip_sb = small.tile([128, 1], F32)
                    nc.vector.reciprocal(recip_sb, ot_ps[:, H : H + 1])
                    nc.vector.tensor_scalar(
                        out=out_acc[:, qt, :],
                        in0=ot_ps[:, 0:H],
                        scalar1=recip_sb,
                        scalar2=maskq_sb[:, qt : qt + 1],
                        op0=mybir.AluOpType.mult,
                        op1=mybir.AluOpType.mult,
                    )
            nc.sync.dma_start(
                out=out_d.ap().rearrange("(n p) h -> p n h", p=128), in_=out_acc
            )


_NC_CACHE = None


def _get_nc():
    global _NC_CACHE
    if _NC_CACHE is None:
        _NC_CACHE = build_nc()
    return _NC_CACHE


def build_nc_reps(reps):
    return build_nc(reps=reps)


def _prep_core_inputs(x, padding_mask, wkt, wqt, wvt, ident, core):
    b, half = core // 2, core % 2
    q0 = half * QL
    xb = x[b]
    m = padding_mask[b, 0].astype(np.float32)
    if half:  # rotate keys so local queries are rows 0:2048 (exact: permutation
        # of keys with identically-permuted key mask leaves attention unchanged)
        xb = np.concatenate([xb[q0:], xb[:q0]], axis=0)
        m = np.concatenate([m[q0:], m[:q0]], axis=0)
    nbias = np.ascontiguousarray((NEG * (1.0 - m)).reshape(NKC, 128).T)
    maskq = np.ascontiguousarray(m[:QL].reshape(QL // 128, 128).T)
    return {
        "x": np.ascontiguousarray(xb),
        "wkt": wkt,
        "wqt": wqt,
        "wvt": wvt,
        "ident": ident,
        "nbias": nbias,
        "maskq": maskq,
    }


def make_in_maps(x, padding_mask, Wk, Wq, Wv):
    def wt(w):  # [64,1024] -> [128, 8, 64]: wt[p, cc, h] = w[h, cc*128+p]
        return np.ascontiguousarray(w.T.reshape(NCC, 128, H).transpose(1, 0, 2))

    wkt, wqt, wvt = wt(np.asarray(Wk)), wt(np.asarray(Wq)), wt(np.asarray(Wv))
    ident = np.eye(128, dtype=np.float32)
    x = np.asarray(x)
    padding_mask = np.asarray(padding_mask)
    return [
        _prep_core_inputs(x, padding_mask, wkt, wqt, wvt, ident, c) for c in range(8)
    ]


def kernel(x, padding_mask, Wk, Wq, Wv):
    nc = _get_nc()
    in_maps = make_in_maps(x, padding_mask, Wk, Wq, Wv)
    res = run_bass_kernel_spmd(nc, in_maps, core_ids=list(range(8)), trace=False)
    B = x.shape[0]
    out = np.empty((B, T, H), dtype=np.float32)
    for c in range(8):
        b, half = c // 2, c % 2
        out[b, half * QL : (half + 1) * QL, :] = res.results[c]["out"]
    return out


# revision 6
# speedup vs baseline: 409.4294x; 3.0623x over previous
"""Single-head attention kernel for Trainium2 (8 NeuronCores, SPMD).

Problem: x[4,4096,1024] f32, padding_mask[4,1,4096] i32, Wk/Wq/Wv[64,1024] f32.
  k/q/v = x @ W.T ; wei = softmax(mask(q k^T / 8)) ; out = wei @ v  -> [4,4096,64]

Sharding: core c handles (batch b = c//2, query half = c%2).  Each core gets the
full x[b] (rotated so its 2048 local queries are always rows 0:2048 -- attention
is permutation-invariant over keys, so rotating keys + key-mask together is
exact), computes k/v for all 4096 keys and q for its half, and returns
out[2048, 64].

Device algorithm (per core):
  Phase 1: transpose x via PE transpose (contraction needs C on partitions),
    then project kT/qT/vT = W.T^T @ xT in [H, T] layout; vT is re-transposed
    into v[keys, 64] and extended with a ones column (col 64) used to compute
    softmax denominators via the second matmul.
  Phase 2: per 512-query block: sT[keys,queries] = kT_chunk.T @ qT (PSUM),
    exp via ScalarE with per-partition bias = -1e5*(1-key_mask) (masked keys
    underflow to exactly 0; no row-max subtraction needed since scores are
    O(5)), then oT[65, 512] += v_ext.T @ exp accumulated over key chunks.
    Epilogue: transpose oT back, scale rows by query_mask/denominator.

All large matmuls use float32r operands (TF32-like, 1 cycle/row vs fp32's 4;
measured rel err ~1.5e-4, far inside the f32 envelope for this softmax).
Producers round explicitly (DVE/ACT writes with f32r output dtype).
"""

import sys

if "/opt/trn_rl_repo" not in sys.path:
    sys.path.insert(0, "/opt/trn_rl_repo")

import numpy as np

import concourse.bass as bass
import concourse.mybir as mybir
import concourse.tile as tile
from concourse import bacc
from concourse.bass_utils import run_bass_kernel_spmd

F32 = mybir.dt.float32
F32R = mybir.dt.float32r
T = 4096          # sequence length (keys)
C = 1024          # embedding dim
H = 64            # head size
TBS = 512         # t-block size for phase 1
NTB = T // TBS    # 8 t-blocks
NCC = C // 128    # 8 c-chunks
QL = 2048         # local queries per core
NQB = QL // 512   # 4 query blocks
NKC = T // 128    # 32 key chunks
NEG = -1.0e5      # masked-key bias: exp(s/8 + NEG) underflows to 0.0


def build_nc(reps=1):
    nc = bacc.Bacc("TRN2", target_bir_lowering=False, debug=False, num_devices=8)

    x_d = nc.dram_tensor("x", [T, C], F32, kind="ExternalInput")
    wkt_d = nc.dram_tensor("wkt", [128, NCC, H], F32, kind="ExternalInput")
    wqt_d = nc.dram_tensor("wqt", [128, NCC, H], F32, kind="ExternalInput")
    wvt_d = nc.dram_tensor("wvt", [128, NCC, H], F32, kind="ExternalInput")
    ident_d = nc.dram_tensor("ident", [128, 128], F32, kind="ExternalInput")
    nbias_d = nc.dram_tensor("nbias", [128, NKC], F32, kind="ExternalInput")
    maskq_d = nc.dram_tensor("maskq", [128, QL // 128], F32, kind="ExternalInput")
    out_d = nc.dram_tensor("out", [QL, H], F32, kind="ExternalOutput")

    with tile.TileContext(nc) as tc:
        with (
            tc.tile_pool(name="const", bufs=1) as const,
            tc.tile_pool(name="persist", bufs=1) as persist,
            tc.tile_pool(name="xin", bufs=2) as xin,
            tc.tile_pool(name="xt", bufs=3) as xtp,
            tc.tile_pool(name="vt", bufs=2) as vtp,
            tc.tile_pool(name="expp", bufs=4) as expp,
            tc.tile_pool(name="osb", bufs=2) as osb,
            tc.tile_pool(name="small", bufs=4) as small,
            tc.tile_pool(name="psA", bufs=2, space=bass.MemorySpace.PSUM) as psA,
            tc.tile_pool(name="psB", bufs=1, space=bass.MemorySpace.PSUM) as psB,
            tc.tile_pool(name="psC", bufs=2, space=bass.MemorySpace.PSUM) as psC,
            tc.tile_pool(name="psD", bufs=1, space=bass.MemorySpace.PSUM) as psD,
        ):
            pools = (const, persist, xin, xtp, vtp, expp, osb, small,
                     psA, psB, psC, psD)
            drams = (x_d, wkt_d, wqt_d, wvt_d, ident_d, nbias_d, maskq_d, out_d)

            if reps == 1:
                _emit(nc, pools, drams)
            else:
                with tc.For_i(0, reps):
                    _emit(nc, pools, drams)

    nc.compile()
    return nc


def _emit(nc, pools, drams):
    (const, persist, xin, xtp, vtp, expp, osb, small,
     psA, psB, psC, psD) = pools
    (x_d, wkt_d, wqt_d, wvt_d, ident_d, nbias_d, maskq_d, out_d) = drams

    # ---- constants ----
    wstage = const.tile([128, 3, NCC, H], F32)
    nc.sync.dma_start(out=wstage[:, 0], in_=wkt_d.ap())
    nc.sync.dma_start(out=wstage[:, 1], in_=wqt_d.ap())
    nc.sync.dma_start(out=wstage[:, 2], in_=wvt_d.ap())
    w_sb = const.tile([128, 3, NCC, H], F32R)   # rounded for f32r matmuls
    nc.vector.tensor_copy(w_sb, wstage)
    wkt_sb, wqt_sb, wvt_sb = w_sb[:, 0], w_sb[:, 1], w_sb[:, 2]
    ident_sb = const.tile([128, 128], F32)
    nbias_sb = const.tile([128, NKC], F32)
    maskq_sb = const.tile([128, QL // 128], F32)
    nc.sync.dma_start(out=ident_sb, in_=ident_d.ap())
    nc.sync.dma_start(out=nbias_sb, in_=nbias_d.ap())
    nc.sync.dma_start(out=maskq_sb, in_=maskq_d.ap())

    # ---- persistent intermediates ----
    kT_sb = persist.tile([H, T], F32R)           # k^T  [64, 4096]
    qT_sb = persist.tile([H, QL], F32R)          # q^T  [64, 2048]
    v_sb = persist.tile([128, NKC, H + 1], F32R)  # v_ext [keys, 65]
    out_acc = persist.tile([128, QL // 128, H], F32)
    ones_sb = const.tile([128, NKC], F32)
    nc.vector.memset(ones_sb, 1.0)
    nc.vector.tensor_copy(v_sb[:, :, H], ones_sb)  # ones column (rounds to f32r)

    # ================= Phase 1: transpose + projections ============
    for tb in range(NTB):
        x_tile = xin.tile([128, TBS // 128, C], F32)
        nc.sync.dma_start(
            out=x_tile,
            in_=x_d.ap()[tb * TBS : (tb + 1) * TBS, :].rearrange(
                "(s p) c -> p s c", p=128
            ),
        )
        kqv_ps = psB.tile([H, 3, TBS], F32)
        for cc in range(NCC):
            # transpose x[tb, cc]: 4x [128t,128c] -> [128c, 512t]
            tp_ps = psA.tile([128, TBS], F32, tag="pa")
            for s in range(TBS // 128):
                nc.tensor.transpose(
                    tp_ps[:, s * 128 : (s + 1) * 128],
                    x_tile[:, s, cc * 128 : (cc + 1) * 128],
                    ident_sb,
                )
            xT_sb = xtp.tile([128, TBS], F32R)
            # alternate ACT/DVE for PSUM->SBUF copies to split the load
            if cc % 2 == 0:
                nc.scalar.copy(xT_sb, tp_ps)
            else:
                nc.vector.tensor_copy(xT_sb, tp_ps)
            first, last = cc == 0, cc == NCC - 1
            nc.tensor.matmul(
                kqv_ps[:, 0, :], wkt_sb[:, cc, :], xT_sb, start=first, stop=last
            )
            nc.tensor.matmul(
                kqv_ps[:, 1, :], wvt_sb[:, cc, :], xT_sb, start=first, stop=last
            )
            if tb < NQB:  # local queries are always rows 0:2048
                nc.tensor.matmul(
                    kqv_ps[:, 2, :], wqt_sb[:, cc, :], xT_sb, start=first, stop=last
                )
        nc.vector.tensor_copy(kT_sb[:, tb * TBS : (tb + 1) * TBS], kqv_ps[:, 0, :])
        if tb < NQB:
            nc.vector.tensor_copy(qT_sb[:, tb * TBS : (tb + 1) * TBS], kqv_ps[:, 2, :])
        # vT -> v (re-transpose to [keys, 64] layout)
        vT_sb = vtp.tile([H, TBS], F32)
        nc.scalar.copy(vT_sb, kqv_ps[:, 1, :])
        vtp_ps = psC.tile([128, TBS // 128, H], F32, tag="small")
        for s in range(TBS // 128):
            nc.tensor.transpose(
                vtp_ps[:, s, :],
                vT_sb[:, s * 128 : (s + 1) * 128],
                ident_sb[:H, :H],
            )
        nc.vector.tensor_copy(
            v_sb[:, tb * (TBS // 128) : (tb + 1) * (TBS // 128), 0:H], vtp_ps
        )

    # ================= Phase 2: attention =========================
    for qb in range(NQB):
        oT_ps = psD.tile([H + 1, 512], F32)
        for kc in range(NKC):
            sT_ps = psA.tile([128, 512], F32, tag="pa")
            nc.tensor.matmul(
                sT_ps,
                kT_sb[:, kc * 128 : (kc + 1) * 128],
                qT_sb[:, qb * 512 : (qb + 1) * 512],
                start=True,
                stop=True,
            )
            exp_sb = expp.tile([128, 512], F32R)
            nc.scalar.activation(
                exp_sb,
                sT_ps,
                mybir.ActivationFunctionType.Exp,
                bias=nbias_sb[:, kc : kc + 1],
                scale=0.125,
            )
            nc.tensor.matmul(
                oT_ps,
                v_sb[:, kc, :],
                exp_sb,
                start=(kc == 0),
                stop=(kc == NKC - 1),
            )
        oT_sb = osb.tile([H + 1, 512], F32)
        nc.vector.tensor_copy(oT_sb, oT_ps)
        for qs in range(4):
            qt = qb * 4 + qs
            ot_ps = psC.tile([128, H + 1], F32, tag="small")
            nc.tensor.transpose(
                ot_ps,
                oT_sb[:, qs * 128 : (qs + 1) * 128],
                ident_sb[: H + 1, : H + 1],
            )
            recip_sb = small.tile([128, 1], F32)
            nc.vector.reciprocal(recip_sb, ot_ps[:, H : H + 1])
            nc.vector.tensor_scalar(
                out=out_acc[:, qt, :],
                in0=ot_ps[:, 0:H],
                scalar1=recip_sb,
                scalar2=maskq_sb[:, qt : qt + 1],
                op0=mybir.AluOpType.mult,
                op1=mybir.AluOpType.mult,
            )
    nc.sync.dma_start(
        out=out_d.ap().rearrange("(n p) h -> p n h", p=128), in_=out_acc
    )


_NC_CACHE = None


def _get_nc():
    global _NC_CACHE
    if _NC_CACHE is None:
        _NC_CACHE = build_nc()
    return _NC_CACHE


def build_nc_reps(reps):
    return build_nc(reps=reps)


def _prep_core_inputs(x, padding_mask, wkt, wqt, wvt, ident, core):
    b, half = core // 2, core % 2
    q0 = half * QL
    xb = x[b]
    m = padding_mask[b, 0].astype(np.float32)
    if half:  # rotate keys so local queries are rows 0:2048 (exact: permutation
        # of keys with identically-permuted key mask leaves attention unchanged)
        xb = np.concatenate([xb[q0:], xb[:q0]], axis=0)
        m = np.concatenate([m[q0:], m[:q0]], axis=0)
    nbias = np.ascontiguousarray((NEG * (1.0 - m)).reshape(NKC, 128).T)
    maskq = np.ascontiguousarray(m[:QL].reshape(QL // 128, 128).T)
    return {
        "x": np.ascontiguousarray(xb),
        "wkt": wkt,
        "wqt": wqt,
        "wvt": wvt,
        "ident": ident,
        "nbias": nbias,
        "maskq": maskq,
    }


def make_in_maps(x, padding_mask, Wk, Wq, Wv):
    def wt(w):  # [64,1024] -> [128, 8, 64]: wt[p, cc, h] = w[h, cc*128+p]
        return np.ascontiguousarray(w.T.reshape(NCC, 128, H).transpose(1, 0, 2))

    wkt, wqt, wvt = wt(np.asarray(Wk)), wt(np.asarray(Wq)), wt(np.asarray(Wv))
    ident = np.eye(128, dtype=np.float32)
    x = np.asarray(x)
    padding_mask = np.asarray(padding_mask)
    return [
        _prep_core_inputs(x, padding_mask, wkt, wqt, wvt, ident, c) for c in range(8)
    ]


def kernel(x, padding_mask, Wk, Wq, Wv):
    nc = _get_nc()
    in_maps = make_in_maps(x, padding_mask, Wk, Wq, Wv)
    res = run_bass_kernel_spmd(nc, in_maps, core_ids=list(range(8)), trace=False)
    B = x.shape[0]
    out = np.empty((B, T, H), dtype=np.float32)
    for c in range(8):
        b, half = c // 2, c % 2
        out[b, half * QL : (half + 1) * QL, :] = res.results[c]["out"]
    return out
